# revision 21
# baseline (speedup 1.0000x reference)
"""Trainium2 Bass kernel for nn_Attention_28802050687173.

Channel-attention block: 1x1 conv (c->4c), depthwise 3x3, gating multiply,
L2-normalized channel gram + softmax, attn @ v, 1x1 conv out.

Sharding: 8 cores = (sample, H-half).  Each core processes 128 rows x 256 cols
of one sample (n_loc = 32768 pixels).  The depthwise conv is folded into the
input projection: dw = sum_j (w_dw[:,j] * W_in1) @ x_shift_j, so the whole
front end is 7 matmuls per tile over a zero-padded, duplicated+shifted copy of
x built host-side.  The channel gram S = [v;q][v;q]^T is accumulated on-chip
(PE transposes + bf16 matmuls) and AllReduced between the two half-sample
cores; softmax and the fused (w_out @ attn) @ v output projection follow.
"""
import numpy as np

import concourse.bass as bass
import concourse.mybir as mybir
import concourse.tile as tile
from concourse import bacc
from concourse.bass_utils import run_bass_kernel_spmd
from concourse.masks import make_identity

F32 = mybir.dt.float32
F32R = mybir.dt.float32r
BF16 = mybir.dt.bfloat16


def _install_ntff_hook():
    """The container's antenv stub lacks axon_hooks, so bass_utils'
    trace=True path can't find the NTFF profile hook the axon .so
    provides.  Recreate the hook (same ctypes ABI trn_agent_boot uses)
    and inject an antenv.axon_hooks module exposing it."""
    import sys
    import contextlib
    import ctypes
    if "antenv.axon_hooks" in sys.modules:
        return
    so_path = "/opt/axon/libaxon_pjrt.so"
    try:
        lib = ctypes.CDLL(so_path)
    except OSError:
        return
    if not hasattr(lib, "axon_start_nrt_profile"):
        return
    lib.axon_start_nrt_profile.argtypes = [
        ctypes.POINTER(ctypes.c_int64), ctypes.c_size_t]
    lib.axon_start_nrt_profile.restype = ctypes.c_int64
    lib.axon_stop_nrt_profile.argtypes = [ctypes.c_char_p]
    lib.axon_stop_nrt_profile.restype = ctypes.c_int64

    @contextlib.contextmanager
    def _hook(output_dir, device_ids):
        import jax
        jax.devices()
        if device_ids:
            ids = (ctypes.c_int64 * len(device_ids))(*device_ids)
            rc = lib.axon_start_nrt_profile(ids, len(device_ids))
        else:
            rc = lib.axon_start_nrt_profile(None, 0)
        if rc != 0:
            raise RuntimeError(f"axon_start_nrt_profile rc={rc}")
        try:
            yield
        finally:
            n = lib.axon_stop_nrt_profile(str(output_dir).encode())
            if n < 0:
                raise RuntimeError(f"axon_stop_nrt_profile rc={n}")

    import types
    mod = types.ModuleType("antenv.axon_hooks")
    mod._hook = _hook
    mod.get_axon_ntff_profile_hook = lambda: mod._hook
    mod.set_axon_ntff_profile_hook = lambda h: setattr(mod, "_hook", h)
    sys.modules["antenv.axon_hooks"] = mod
    try:
        import antenv
        antenv.axon_hooks = mod
    except ImportError:
        pass

B, C, H, W = 4, 64, 256, 256
RS = 258                     # zero-padded row stride
HROWS = 130                  # 128 output rows + 1 halo row each side
XLEN = HROWS * RS            # 33540 elements per channel per core
SHIFT = 259                  # dup-half shift: tap (ky,kx) -> (ky+1,kx+1)
N = 128 * 256                # 32768 outputs per core
NT = 512                     # matmul/psum tile (2 output rows)
WINR = 8                     # output rows per DMA window
WIN = (WINR + 2) * RS        # 2580 elements per window
NWIN = 128 // WINR           # 16 windows
SUBT = WINR // 2             # 4 sub-tiles per window
PAIR_TAPS = [(0, 0), (0, 1), (1, 0)]     # (ky,kx); partner = (ky+1,kx+1)
SINGLE_TAPS = [(0, 2), (2, 0), (2, 2)]
EPS = 1e-12
RG = [[0, 1], [2, 3], [4, 5], [6, 7]]    # AllReduce pairs (same sample)

_CACHE = {}


def _rhs3(xd_t, parts, j, ky, kx, p0=0):
    """[parts, 2, 256] view: output sub-tile j, tap (ky, kx)."""
    v = xd_t[p0:p0 + parts, :].rearrange("p (r c) -> p r c", r=WINR + 2, c=RS)
    return v[:, 2 * j + ky: 2 * j + ky + 2, kx: kx + 256]


def build_nc():
    nc = bacc.Bacc("TRN2", target_bir_lowering=False, debug=False, num_devices=8)

    xd_d = nc.dram_tensor("xd", [128, XLEN], F32R, kind="ExternalInput")
    wp_d = nc.dram_tensor("wp", [128, 3 * 128], F32R, kind="ExternalInput")
    ws_d = nc.dram_tensor("ws", [64, 3 * 128], F32R, kind="ExternalInput")
    w2_d = nc.dram_tensor("w2", [64, 128], F32R, kind="ExternalInput")
    wo_d = nc.dram_tensor("wo", [64, 64], F32, kind="ExternalInput")
    tp_d = nc.dram_tensor("tp", [1, 1], F32, kind="ExternalInput")
    out_d = nc.dram_tensor("out", [64, N], F32, kind="ExternalOutput")

    with tile.TileContext(nc) as tc:
        from contextlib import ExitStack
        with ExitStack() as outer:
            pool_w = outer.enter_context(tc.tile_pool(name="wts", bufs=1))
            pool_s = outer.enter_context(tc.tile_pool(name="sbuf_s", bufs=1))
            pool_ps_S = outer.enter_context(
                tc.tile_pool(name="ps_S", bufs=1, space="PSUM"))
            pool_dram = outer.enter_context(
                tc.tile_pool(name="dram", bufs=1, space="DRAM"))

            # persistent tiles
            wp_sb = pool_w.tile([128, 3 * 128], F32R)
            ws_sb = pool_w.tile([64, 3 * 128], F32R)
            w2_sb = pool_w.tile([64, 128], F32R)
            wo_sb = pool_w.tile([64, 64], F32)
            tp_sb = pool_w.tile([1, 1], F32)
            id_bf = pool_w.tile([128, 128], BF16)
            id_f32 = pool_w.tile([64, 64], F32)
            ones_sb = pool_w.tile([1, 64], F32)
            s_t = pool_s.tile([128, N], BF16)
            S_ps = pool_ps_S.tile([128, 128], F32)
            cc_in = pool_dram.tile([129, 128], F32)
            cc_out = pool_dram.tile([129, 128], F32)
            cc_in_b = pool_dram.tile([129, 128], F32)
            cc_out_b = pool_dram.tile([129, 128], F32)

            nc.sync.dma_start(wp_sb[:], wp_d[:])
            nc.sync.dma_start(ws_sb[:], ws_d[:])
            nc.sync.dma_start(w2_sb[:], w2_d[:])
            nc.sync.dma_start(wo_sb[:], wo_d[:])
            nc.sync.dma_start(tp_sb[:], tp_d[:])
            make_identity(nc, id_bf[:])
            make_identity(nc, id_f32[:])
            nc.gpsimd.memset(ones_sb[:], 1.0)
            # preload ACT table sets (sqrt, exp) so the softmax phase does
            # not pay the ~2.7us-per-set load inside the collective gap
            scr_a = pool_w.tile([1, 1], F32)
            scr_b = pool_w.tile([1, 1], F32)
            nc.scalar.sqrt(scr_a[:], tp_sb[:])
            nc.scalar.activation(scr_b[:], scr_a[:],
                                 mybir.ActivationFunctionType.Exp)
            # constant f32 diag mask (expanded from bf16 identity)
            diag_tmp = pool_w.tile([128, 128], F32)
            nc.scalar.copy(diag_tmp[:], id_bf[:])

            # ---------------- pass 1: conv front-end + gram ----------------
            NTILES = N // NT
            SPLIT = 56          # tiles [0, SPLIT) -> S_a, rest -> S_b
            S_ps_b = pool_ps_S.tile([128, 128], F32)
            Sa_sb = pool_w.tile([128, 128], F32)
            diag_a = pool_w.tile([128, 1], F32)
            dtmp_a = pool_w.tile([128, 128], F32)
            # warm the PE HAM before pass 1: a dense burst of dummy
            # matmuls with (almost) no dependencies that runs during the
            # initial DMA waits
            with tc.tile_pool(name="ps_w0", bufs=1, space="PSUM") as pw0:
                warm0 = pw0.tile([128, 128], F32)
                for _ in range(70):
                    nc.tensor.matmul(warm0[:], id_bf[:], id_bf[:],
                                     start=True, stop=True)

            with ExitStack() as p1:
                pool_xd = p1.enter_context(tc.tile_pool(name="xd", bufs=2))
                pool_dw = p1.enter_context(
                    tc.tile_pool(name="ps_dw", bufs=2, space="PSUM"))
                pool_x2 = p1.enter_context(
                    tc.tile_pool(name="ps_x2", bufs=2, space="PSUM"))
                pool_tr = p1.enter_context(
                    tc.tile_pool(name="ps_tr", bufs=2, space="PSUM"))
                pool_x2sb = p1.enter_context(tc.tile_pool(name="x2sb", bufs=3))
                pool_st = p1.enter_context(tc.tile_pool(name="stsb", bufs=4))

                sT_tiles = {}

                def emit_transpose(t):
                    tr_ps = pool_tr.tile([128, NT], BF16)
                    for q in range(4):
                        nc.tensor.transpose(
                            tr_ps[:, 128 * q: 128 * (q + 1)],
                            s_t[:, NT * t + 128 * q: NT * t + 128 * (q + 1)],
                            id_bf[:])
                    sT_sb = pool_st.tile([128, NT], BF16)
                    nc.vector.tensor_copy(sT_sb[:], tr_ps[:])
                    sT_tiles[t] = sT_sb

                def emit_gram(t):
                    sT_sb = sT_tiles.pop(t)
                    Sdst = S_ps if t < SPLIT else S_ps_b
                    for q in range(4):
                        a = sT_sb[:, 128 * q: 128 * (q + 1)]
                        nc.tensor.matmul(
                            Sdst[:], a, a,
                            start=(t in (0, SPLIT) and q == 0),
                            stop=(t in (SPLIT - 1, NTILES - 1) and q == 3))
                    if t == SPLIT - 1:
                        # evacuate partial gram S_a and start its
                        # AllReduce under the tail of pass 1
                        nc.vector.tensor_copy(Sa_sb[:], S_ps[:])
                        nc.vector.tensor_mul(
                            dtmp_a[:], Sa_sb[:], diag_tmp[:])
                        nc.vector.tensor_reduce(
                            diag_a[:], dtmp_a[:],
                            axis=mybir.AxisListType.X,
                            op=mybir.AluOpType.add)
                        nc.sync.dma_start(cc_in[0:128, :], Sa_sb[:])
                        nc.sync.dma_start(cc_in[128:129, :], diag_a[:])
                        nc.gpsimd.collective_compute(
                            "AllReduce", mybir.AluOpType.add,
                            replica_groups=RG,
                            ins=[cc_in.opt()], outs=[cc_out.opt()])

                for w in range(NWIN):
                    xd_t = pool_xd.tile([128, WIN], F32R)
                    base = w * WINR * RS
                    if w == 0:
                        nc.sync.dma_start(
                            xd_t[:, 0:1032], xd_d[:, base: base + 1032])
                        nc.sync.dma_start(
                            xd_t[:, 1032:WIN], xd_d[:, base + 1032: base + WIN])
                    else:
                        nc.sync.dma_start(
                            xd_t[:], xd_d[:, base: base + WIN])
                    for j in range(SUBT):
                        t = SUBT * w + j
                        # x2 first: its ACT evacuation overlaps the conv MMs
                        x2_ps = pool_x2.tile([128, NT], F32)
                        nc.tensor.matmul(
                            x2_ps[:], w2_sb[:],
                            _rhs3(xd_t, 64, j, 1, 1),
                            start=True, stop=True)
                        x2_sb = pool_x2sb.tile([128, NT], F32)
                        nc.scalar.copy(x2_sb[:], x2_ps[:])
                        dw_ps = pool_dw.tile([128, NT], F32)
                        for p, (ky, kx) in enumerate(PAIR_TAPS):
                            rhs = _rhs3(xd_t, 128, j, ky, kx)
                            nc.tensor.matmul(
                                dw_ps[:],
                                wp_sb[:, 128 * p: 128 * (p + 1)],
                                rhs,
                                start=(p == 0), stop=False)
                        for si, (ky, kx) in enumerate(SINGLE_TAPS):
                            nc.tensor.matmul(
                                dw_ps[:],
                                ws_sb[:, 128 * si: 128 * (si + 1)],
                                _rhs3(xd_t, 64, j, ky, kx),
                                start=False, stop=(si == 2))
                        # PE fills the wait for this tile's DVE mult with
                        # last tile's transposes and an older gram
                        if t >= 1:
                            emit_transpose(t - 1)
                        if t >= 2:
                            emit_gram(t - 2)
                        nc.vector.tensor_mul(
                            s_t[:, NT * t: NT * (t + 1)], dw_ps[:], x2_sb[:])
                emit_transpose(NTILES - 1)
                emit_gram(NTILES - 2)
                emit_gram(NTILES - 1)

            # ---------------- second (small) gram AllReduce ----------------
            Sb_sb = pool_w.tile([128, 128], F32)
            diag_b = pool_w.tile([128, 1], F32)
            nc.vector.tensor_copy(Sb_sb[:], S_ps_b[:])
            nc.vector.tensor_mul(diag_tmp[:], S_ps_b[:], diag_tmp[:])
            nc.vector.tensor_reduce(
                diag_b[:], diag_tmp[:], axis=mybir.AxisListType.X,
                op=mybir.AluOpType.add)
            nc.sync.dma_start(cc_in_b[0:128, :], Sb_sb[:])
            nc.sync.dma_start(cc_in_b[128:129, :], diag_b[:])
            nc.gpsimd.collective_compute(
                "AllReduce", mybir.AluOpType.add, replica_groups=RG,
                ins=[cc_in_b.opt()], outs=[cc_out_b.opt()])

            # keep the PE HAM-warm through the collective gap: dummy bf16
            # matmuls reading s tiles (they depend on Sb_sb so they cannot
            # start before pass 1 ends)
            warm_ps_pool = tc.tile_pool(name="ps_warm", bufs=1, space="PSUM")
            with warm_ps_pool as pw:
                warm_ps = pw.tile([64, 128], F32)
                for _ in range(140):
                    nc.tensor.matmul(warm_ps[:], Sb_sb[0:64, 0:64],
                                     Sb_sb[0:64, :], start=True, stop=True)

            # readback: sum the two partial AllReduce results
            gvq_a2 = pool_w.tile([64, 64], F32)
            gvq_b2 = pool_w.tile([64, 64], F32)
            sq_a2 = pool_w.tile([64, 1], F32)
            sq_b2 = pool_w.tile([64, 1], F32)
            sv_a2 = pool_w.tile([1, 64], F32)
            sv_b2 = pool_w.tile([1, 64], F32)
            nc.sync.dma_start(gvq_a2[:], cc_out[0:64, 64:128])
            nc.sync.dma_start(sq_a2[:], cc_out[128:129, 64:128])
            nc.sync.dma_start(sv_a2[:], cc_out[128:129, 0:64])
            nc.sync.dma_start(gvq_b2[:], cc_out_b[0:64, 64:128])
            nc.sync.dma_start(sq_b2[:], cc_out_b[128:129, 64:128])
            nc.sync.dma_start(sv_b2[:], cc_out_b[128:129, 0:64])
            gvq_sb = pool_w.tile([64, 64], F32)     # [d, c] = v_d . q_c
            sq_sb = pool_w.tile([64, 1], F32)
            sv_sb = pool_w.tile([1, 64], F32)
            nc.vector.tensor_add(gvq_sb[:], gvq_a2[:], gvq_b2[:])
            nc.vector.tensor_add(sq_sb[:], sq_a2[:], sq_b2[:])
            nc.vector.tensor_add(sv_sb[:], sv_a2[:], sv_b2[:])

            # ---------------- softmax + fused output weights ----------------
            with ExitStack() as p15:
                ps_sm = p15.enter_context(
                    tc.tile_pool(name="ps_sm", bufs=1, space="PSUM"))
                # rq = temp / max(sqrt(sq), eps)   [64, 1]
                nq = pool_w.tile([64, 1], F32)
                nc.scalar.sqrt(nq[:], sq_sb[:])
                nc.vector.tensor_scalar_max(nq[:], nq[:], EPS)
                rq = pool_w.tile([64, 1], F32)
                nc.vector.reciprocal(rq[:], nq[:])
                tb_ps = ps_sm.tile([64, 1], F32)
                nc.tensor.matmul(tb_ps[:], ones_sb[:], tp_sb[:],
                                 start=True, stop=True)
                nc.vector.tensor_mul(rq[:], rq[:], tb_ps[:])
                # rv row then broadcast to [64, 64] via ones-column matmul
                nv = pool_w.tile([1, 64], F32)
                nc.scalar.sqrt(nv[:], sv_sb[:])
                nc.vector.tensor_scalar_max(nv[:], nv[:], EPS)
                rv = pool_w.tile([1, 64], F32)
                nc.vector.reciprocal(rv[:], nv[:])
                rvb_ps = ps_sm.tile([64, 64], F32)
                nc.tensor.matmul(rvb_ps[:], ones_sb[:], rv[:],
                                 start=True, stop=True)
                # Gqv = Gvq^T
                gqv_ps = ps_sm.tile([64, 64], F32)
                nc.tensor.transpose(gqv_ps[:], gvq_sb[:], id_f32[:])
                gqv_sb = pool_w.tile([64, 64], F32)
                nc.vector.tensor_copy(gqv_sb[:], gqv_ps[:])
                # z = Gqv * rq[c] * rv[d]
                z = pool_w.tile([64, 64], F32)
                nc.vector.scalar_tensor_tensor(
                    out=z[:], in0=gqv_sb[:], scalar=rq[:], in1=rvb_ps[:],
                    op0=mybir.AluOpType.mult, op1=mybir.AluOpType.mult)
                mx = pool_w.tile([64, 1], F32)
                nc.vector.tensor_reduce(
                    mx[:], z[:], axis=mybir.AxisListType.X,
                    op=mybir.AluOpType.max)
                nc.vector.tensor_scalar(
                    out=z[:], in0=z[:], scalar1=mx[:], scalar2=None,
                    op0=mybir.AluOpType.subtract)
                e = pool_w.tile([64, 64], F32)
                sums = pool_w.tile([64, 1], F32)
                nc.scalar.activation(
                    e[:], z[:], mybir.ActivationFunctionType.Exp,
                    accum_out=sums[:])
                rs = pool_w.tile([64, 1], F32)
                nc.vector.reciprocal(rs[:], sums[:])
                attn = pool_w.tile([64, 64], F32)
                nc.vector.tensor_scalar(
                    out=attn[:], in0=e[:], scalar1=rs[:], scalar2=None,
                    op0=mybir.AluOpType.mult)
                # A2T = attn^T @ w_out^T  ->  [d, o]
                a2t_ps = ps_sm.tile([64, 64], F32)
                nc.tensor.matmul(a2t_ps[:], attn[:], wo_sb[:],
                                 start=True, stop=True)
                a2t_bf = pool_w.tile([64, 64], BF16)
                nc.vector.tensor_copy(a2t_bf[:], a2t_ps[:])

            # ---------------- pass 2: out = A2 @ v, streamed ----------------
            with ExitStack() as p2:
                ps_o = p2.enter_context(
                    tc.tile_pool(name="ps_o", bufs=3, space="PSUM"))
                ob_pool = p2.enter_context(tc.tile_pool(name="ob", bufs=3))
                BIG = 4096
                for T in range(N // BIG):
                    ob_sb = ob_pool.tile([128, BIG // 2], F32)
                    for j in range(4):
                        k = (BIG // NT) * T + 2 * j
                        ps = ps_o.tile([128, NT], F32)
                        nc.tensor.matmul(
                            ps[0:64, :], a2t_bf[:],
                            s_t[0:64, NT * k: NT * (k + 1)],
                            start=True, stop=True)
                        nc.tensor.matmul(
                            ps[64:128, :], a2t_bf[:],
                            s_t[0:64, NT * (k + 1): NT * (k + 2)],
                            start=True, stop=True, tile_position=(0, 64))
                        if j % 2 == 0:
                            nc.scalar.copy(
                                ob_sb[:, NT * j: NT * (j + 1)], ps[:])
                        else:
                            nc.vector.tensor_copy(
                                ob_sb[:, NT * j: NT * (j + 1)], ps[:])
                    dstv = out_d[0:64, BIG * T: BIG * (T + 1)].rearrange(
                        "c (j f) -> c j f", j=4, f=2 * NT)
                    srcv = ob_sb[:].rearrange("p (j f) -> p j f", j=4, f=NT)
                    nc.sync.dma_start(dstv[:, :, 0:NT], srcv[0:64])
                    nc.sync.dma_start(dstv[:, :, NT:2 * NT], srcv[64:128])

    nc.compile()
    return nc


def _get_nc():
    if "nc" not in _CACHE:
        _CACHE["nc"] = build_nc()
    return _CACHE["nc"]


def _prep_in_maps(x, w_in, w_dw, w_out, temperature):
    x = np.ascontiguousarray(x, dtype=np.float32)
    w_in = np.asarray(w_in, dtype=np.float32)
    w_dw = np.asarray(w_dw, dtype=np.float32)
    w_out = np.asarray(w_out, dtype=np.float32)
    temp = np.asarray(temperature, dtype=np.float32).reshape(1, 1)

    perm = np.concatenate([np.arange(64, 128), np.arange(0, 64)])
    W_in1 = w_in[:2 * C]          # [128, 64]
    W_in2 = w_in[2 * C:]          # [128, 64]
    wd = w_dw[:, 0]               # [128, 3, 3]

    wp = np.empty((128, 3 * 128), dtype=np.float32)
    for p, (ky, kx) in enumerate(PAIR_TAPS):
        wp[:64, 128 * p:128 * (p + 1)] = \
            (W_in1[perm] * wd[perm, ky, kx][:, None]).T
        wp[64:, 128 * p:128 * (p + 1)] = \
            (W_in1[perm] * wd[perm, ky + 1, kx + 1][:, None]).T
    ws = np.empty((64, 3 * 128), dtype=np.float32)
    for si, (ky, kx) in enumerate(SINGLE_TAPS):
        ws[:, 128 * si:128 * (si + 1)] = \
            (W_in1[perm] * wd[perm, ky, kx][:, None]).T
    w2 = np.ascontiguousarray(W_in2[perm].T)        # [64, 128]
    wo = np.ascontiguousarray(w_out.T)              # [64, 64]

    in_maps = []
    for core in range(8):
        sample, half = core // 2, core % 2
        h0 = half * 128
        xp = np.zeros((64, HROWS, RS), dtype=np.float32)
        lo, hi = max(h0 - 1, 0), min(h0 + 129, H)
        xp[:, lo - (h0 - 1): hi - (h0 - 1), 1:257] = x[sample, :, lo:hi, :]
        xpf = xp.reshape(64, XLEN)
        xpf_pad = np.pad(xpf, ((0, 0), (0, SHIFT)))
        xd = np.concatenate(
            [xpf, xpf_pad[:, SHIFT:SHIFT + XLEN]], axis=0)
        in_maps.append({
            "xd": np.ascontiguousarray(xd), "wp": wp, "ws": ws,
            "w2": w2, "wo": wo, "tp": temp,
        })
    return in_maps


def _assemble(results):
    out = np.empty((B, C, H, W), dtype=np.float32)
    for core in range(8):
        sample, half = core // 2, core % 2
        out[sample, :, half * 128: half * 128 + 128, :] = \
            results[core]["out"].reshape(C, 128, W)
    return out


def run(trace=False, trace_cores=None, **inputs):
    if trace:
        _install_ntff_hook()
    nc = _get_nc()
    in_maps = _prep_in_maps(**inputs)
    res = run_bass_kernel_spmd(nc, in_maps, core_ids=list(range(8)),
                               trace=trace, trace_cores=trace_cores)
    if trace and res.mean_exec_time_ns:
        print(f"mean exec {res.mean_exec_time_ns/1000:.1f}us, "
              f"max core {res.max_exec_time_core_id}")
    return _assemble(res.results), res.exec_time_ns


def kernel(**inputs) -> np.ndarray:
    out, _ = run(trace=False, **inputs)
    return out


# revision 23
# speedup vs baseline: 1.6820x; 1.6820x over previous
"""Trainium2 Bass kernel for nn_Attention_28802050687173.

Channel-attention block: 1x1 conv (c->4c), depthwise 3x3, gating multiply,
L2-normalized channel gram + softmax, attn @ v, 1x1 conv out.

Sharding: 8 cores = (sample, H-half).  Each core processes 128 rows x 256 cols
of one sample (n_loc = 32768 pixels).  The depthwise conv is folded into the
input projection: dw = sum_j (w_dw[:,j] * W_in1) @ x_shift_j, so the whole
front end is 7 matmuls per tile over a zero-padded, duplicated+shifted copy of
x built host-side.  The channel gram S = [v;q][v;q]^T is accumulated on-chip
(PE transposes + bf16 matmuls) and AllReduced between the two half-sample
cores; softmax and the fused (w_out @ attn) @ v output projection follow.
"""
import numpy as np

import concourse.bass as bass
import concourse.mybir as mybir
import concourse.tile as tile
from concourse import bacc
from concourse.bass_utils import run_bass_kernel_spmd
from concourse.masks import make_identity

F32 = mybir.dt.float32
F32R = mybir.dt.float32r
BF16 = mybir.dt.bfloat16


def _install_ntff_hook():
    """The container's antenv stub lacks axon_hooks, so bass_utils'
    trace=True path can't find the NTFF profile hook the axon .so
    provides.  Recreate the hook (same ctypes ABI trn_agent_boot uses)
    and inject an antenv.axon_hooks module exposing it."""
    import sys
    import contextlib
    import ctypes
    if "antenv.axon_hooks" in sys.modules:
        return
    so_path = "/opt/axon/libaxon_pjrt.so"
    try:
        lib = ctypes.CDLL(so_path)
    except OSError:
        return
    if not hasattr(lib, "axon_start_nrt_profile"):
        return
    lib.axon_start_nrt_profile.argtypes = [
        ctypes.POINTER(ctypes.c_int64), ctypes.c_size_t]
    lib.axon_start_nrt_profile.restype = ctypes.c_int64
    lib.axon_stop_nrt_profile.argtypes = [ctypes.c_char_p]
    lib.axon_stop_nrt_profile.restype = ctypes.c_int64

    @contextlib.contextmanager
    def _hook(output_dir, device_ids):
        import jax
        jax.devices()
        if device_ids:
            ids = (ctypes.c_int64 * len(device_ids))(*device_ids)
            rc = lib.axon_start_nrt_profile(ids, len(device_ids))
        else:
            rc = lib.axon_start_nrt_profile(None, 0)
        if rc != 0:
            raise RuntimeError(f"axon_start_nrt_profile rc={rc}")
        try:
            yield
        finally:
            n = lib.axon_stop_nrt_profile(str(output_dir).encode())
            if n < 0:
                raise RuntimeError(f"axon_stop_nrt_profile rc={n}")

    import types
    mod = types.ModuleType("antenv.axon_hooks")
    mod._hook = _hook
    mod.get_axon_ntff_profile_hook = lambda: mod._hook
    mod.set_axon_ntff_profile_hook = lambda h: setattr(mod, "_hook", h)
    sys.modules["antenv.axon_hooks"] = mod
    try:
        import antenv
        antenv.axon_hooks = mod
    except ImportError:
        pass

B, C, H, W = 4, 64, 256, 256
RS = 258                     # zero-padded row stride
HROWS = 130                  # 128 output rows + 1 halo row each side
XLEN = HROWS * RS            # 33540 elements per channel per core
SHIFT = 259                  # dup-half shift: tap (ky,kx) -> (ky+1,kx+1)
N = 128 * 256                # 32768 outputs per core
NT = 512                     # matmul/psum tile (2 output rows)
WINR = 8                     # output rows per DMA window
WIN = (WINR + 2) * RS        # 2580 elements per window
NWIN = 128 // WINR           # 16 windows
SUBT = WINR // 2             # 4 sub-tiles per window
PAIR_TAPS = [(0, 0), (0, 1), (1, 0)]     # (ky,kx); partner = (ky+1,kx+1)
SINGLE_TAPS = [(0, 2), (2, 0), (2, 2)]
EPS = 1e-12
RG = [[0, 1], [2, 3], [4, 5], [6, 7]]    # AllReduce pairs (same sample)

_CACHE = {}


def _rhs3(xd_t, parts, j, ky, kx, p0=0):
    """[parts, 2, 256] view: output sub-tile j, tap (ky, kx)."""
    v = xd_t[p0:p0 + parts, :].rearrange("p (r c) -> p r c", r=WINR + 2, c=RS)
    return v[:, 2 * j + ky: 2 * j + ky + 2, kx: kx + 256]


def build_nc():
    nc = bacc.Bacc("TRN2", target_bir_lowering=False, debug=False, num_devices=8)

    xd_d = nc.dram_tensor("xd", [128, XLEN], BF16, kind="ExternalInput")
    xd3_d = nc.dram_tensor("xd3", [128, XLEN], BF16, kind="ExternalInput")
    wp_d = nc.dram_tensor("wp", [128, 3 * 128], BF16, kind="ExternalInput")
    ws_d = nc.dram_tensor("ws", [128, 2 * 128], BF16, kind="ExternalInput")
    w2_d = nc.dram_tensor("w2", [64, 128], BF16, kind="ExternalInput")
    wo_d = nc.dram_tensor("wo", [64, 64], F32, kind="ExternalInput")
    tp_d = nc.dram_tensor("tp", [1, 1], F32, kind="ExternalInput")
    out_d = nc.dram_tensor("out", [64, N], F32, kind="ExternalOutput")

    with tile.TileContext(nc) as tc:
        from contextlib import ExitStack
        with ExitStack() as outer:
            pool_w = outer.enter_context(tc.tile_pool(name="wts", bufs=1))
            pool_s = outer.enter_context(tc.tile_pool(name="sbuf_s", bufs=1))
            pool_ps_S = outer.enter_context(
                tc.tile_pool(name="ps_S", bufs=1, space="PSUM"))
            pool_dram = outer.enter_context(
                tc.tile_pool(name="dram", bufs=1, space="DRAM"))

            # persistent tiles
            wp_sb = pool_w.tile([128, 3 * 128], BF16)
            ws_sb = pool_w.tile([128, 2 * 128], BF16)
            w2_sb = pool_w.tile([64, 128], BF16)
            wo_sb = pool_w.tile([64, 64], F32)
            tp_sb = pool_w.tile([1, 1], F32)
            id_bf = pool_w.tile([128, 128], BF16)
            id_f32 = pool_w.tile([64, 64], F32)
            ones_sb = pool_w.tile([1, 64], F32)
            s_t = pool_s.tile([128, N], BF16)
            S_ps = pool_ps_S.tile([128, 128], F32)
            cc_in = pool_dram.tile([129, 128], F32)
            cc_out = pool_dram.tile([129, 128], F32)
            cc_in_b = pool_dram.tile([129, 128], F32)
            cc_out_b = pool_dram.tile([129, 128], F32)

            nc.sync.dma_start(wp_sb[:], wp_d[:])
            nc.sync.dma_start(ws_sb[:], ws_d[:])
            nc.sync.dma_start(w2_sb[:], w2_d[:])
            nc.sync.dma_start(wo_sb[:], wo_d[:])
            nc.sync.dma_start(tp_sb[:], tp_d[:])
            make_identity(nc, id_bf[:])
            make_identity(nc, id_f32[:])
            nc.gpsimd.memset(ones_sb[:], 1.0)
            # preload ACT table sets (sqrt, exp) so the softmax phase does
            # not pay the ~2.7us-per-set load inside the collective gap
            scr_a = pool_w.tile([1, 1], F32)
            scr_b = pool_w.tile([1, 1], F32)
            nc.scalar.sqrt(scr_a[:], tp_sb[:])
            nc.scalar.activation(scr_b[:], scr_a[:],
                                 mybir.ActivationFunctionType.Exp)
            # constant f32 diag mask (expanded from bf16 identity)
            diag_tmp = pool_w.tile([128, 128], F32)
            nc.scalar.copy(diag_tmp[:], id_bf[:])

            # ---------------- pass 1: conv front-end + gram ----------------
            NTILES = N // NT
            SPLIT = 56          # tiles [0, SPLIT) -> S_a, rest -> S_b
            S_ps_b = pool_ps_S.tile([128, 128], F32)
            Sa_sb = pool_w.tile([128, 128], F32)
            diag_a = pool_w.tile([128, 1], F32)
            dtmp_a = pool_w.tile([128, 128], F32)
            # warm the PE HAM before pass 1: a dense burst of dummy
            # matmuls with (almost) no dependencies that runs during the
            # initial DMA waits
            with tc.tile_pool(name="ps_w0", bufs=1, space="PSUM") as pw0:
                warm0 = pw0.tile([128, 128], F32)
                for _ in range(12):
                    nc.tensor.matmul(warm0[:], id_bf[:], id_bf[:],
                                     start=True, stop=True)

            with ExitStack() as p1:
                pool_xd = p1.enter_context(tc.tile_pool(name="xd", bufs=2))
                pool_dw = p1.enter_context(
                    tc.tile_pool(name="ps_dw", bufs=2, space="PSUM"))
                pool_x2 = p1.enter_context(
                    tc.tile_pool(name="ps_x2", bufs=2, space="PSUM"))
                pool_tr = p1.enter_context(
                    tc.tile_pool(name="ps_tr", bufs=2, space="PSUM"))
                pool_x2sb = p1.enter_context(tc.tile_pool(name="x2sb", bufs=3))
                pool_st = p1.enter_context(tc.tile_pool(name="stsb", bufs=4))

                sT_tiles = {}

                def emit_transpose(t):
                    tr_ps = pool_tr.tile([128, NT], BF16)
                    for q in range(4):
                        nc.tensor.transpose(
                            tr_ps[:, 128 * q: 128 * (q + 1)],
                            s_t[:, NT * t + 128 * q: NT * t + 128 * (q + 1)],
                            id_bf[:])
                    sT_sb = pool_st.tile([128, NT], BF16)
                    nc.vector.tensor_copy(sT_sb[:], tr_ps[:])
                    sT_tiles[t] = sT_sb

                def emit_gram(t):
                    sT_sb = sT_tiles.pop(t)
                    Sdst = S_ps if t < SPLIT else S_ps_b
                    for q in range(4):
                        a = sT_sb[:, 128 * q: 128 * (q + 1)]
                        nc.tensor.matmul(
                            Sdst[:], a, a,
                            start=(t in (0, SPLIT) and q == 0),
                            stop=(t in (SPLIT - 1, NTILES - 1) and q == 3))
                    if t == SPLIT - 1:
                        # evacuate partial gram S_a and start its
                        # AllReduce under the tail of pass 1
                        nc.vector.tensor_copy(Sa_sb[:], S_ps[:])
                        nc.vector.tensor_mul(
                            dtmp_a[:], Sa_sb[:], diag_tmp[:])
                        nc.vector.tensor_reduce(
                            diag_a[:], dtmp_a[:],
                            axis=mybir.AxisListType.X,
                            op=mybir.AluOpType.add)
                        nc.sync.dma_start(cc_in[0:128, :], Sa_sb[:])
                        nc.sync.dma_start(cc_in[128:129, :], diag_a[:])
                        nc.gpsimd.collective_compute(
                            "AllReduce", mybir.AluOpType.add,
                            replica_groups=RG,
                            ins=[cc_in.opt()], outs=[cc_out.opt()])

                for w in range(NWIN):
                    xd_t = pool_xd.tile([128, WIN], BF16)
                    xd3_t = pool_xd.tile([128, WIN], BF16)
                    base = w * WINR * RS
                    if w == 0:
                        nc.sync.dma_start(
                            xd_t[:, 0:1032], xd_d[:, base: base + 1032])
                        nc.sync.dma_start(
                            xd_t[:, 1032:WIN], xd_d[:, base + 1032: base + WIN])
                    else:
                        nc.sync.dma_start(
                            xd_t[:], xd_d[:, base: base + WIN])
                    nc.sync.dma_start(
                        xd3_t[:], xd3_d[:, base: base + WIN])
                    for j in range(SUBT):
                        t = SUBT * w + j
                        # x2 first: its ACT evacuation overlaps the conv MMs
                        x2_ps = pool_x2.tile([128, NT], F32)
                        nc.tensor.matmul(
                            x2_ps[:], w2_sb[:],
                            _rhs3(xd_t, 64, j, 1, 1),
                            start=True, stop=True)
                        x2_sb = pool_x2sb.tile([128, NT], F32)
                        nc.scalar.copy(x2_sb[:], x2_ps[:])
                        dw_ps = pool_dw.tile([128, NT], F32)
                        for p, (ky, kx) in enumerate(PAIR_TAPS):
                            rhs = _rhs3(xd_t, 128, j, ky, kx)
                            nc.tensor.matmul(
                                dw_ps[:],
                                wp_sb[:, 128 * p: 128 * (p + 1)],
                                rhs,
                                start=(p == 0), stop=False)
                        # xd3 low half = x+516 (tap (2,0)), high = x+2
                        # (tap (0,2)): one K=128 pair, then (2,2) single
                        nc.tensor.matmul(
                            dw_ps[:], ws_sb[:, 0:128],
                            _rhs3(xd3_t, 128, j, 0, 0),
                            start=False, stop=False)
                        nc.tensor.matmul(
                            dw_ps[:], ws_sb[0:64, 128:256],
                            _rhs3(xd3_t, 64, j, 0, 2),
                            start=False, stop=True)
                        # PE fills the wait for this tile's DVE mult with
                        # last tile's transposes and an older gram
                        if t >= 1:
                            emit_transpose(t - 1)
                        if t >= 2:
                            emit_gram(t - 2)
                        nc.vector.tensor_mul(
                            s_t[:, NT * t: NT * (t + 1)], dw_ps[:], x2_sb[:])
                emit_transpose(NTILES - 1)
                emit_gram(NTILES - 2)
                emit_gram(NTILES - 1)

            # ---------------- second (small) gram AllReduce ----------------
            Sb_sb = pool_w.tile([128, 128], F32)
            diag_b = pool_w.tile([128, 1], F32)
            nc.vector.tensor_copy(Sb_sb[:], S_ps_b[:])
            nc.vector.tensor_mul(diag_tmp[:], S_ps_b[:], diag_tmp[:])
            nc.vector.tensor_reduce(
                diag_b[:], diag_tmp[:], axis=mybir.AxisListType.X,
                op=mybir.AluOpType.add)
            nc.sync.dma_start(cc_in_b[0:128, :], Sb_sb[:])
            nc.sync.dma_start(cc_in_b[128:129, :], diag_b[:])
            nc.gpsimd.collective_compute(
                "AllReduce", mybir.AluOpType.add, replica_groups=RG,
                ins=[cc_in_b.opt()], outs=[cc_out_b.opt()])


            # readback: sum the two partial AllReduce results
            gvq_a2 = pool_w.tile([64, 64], F32)
            gvq_b2 = pool_w.tile([64, 64], F32)
            sq_a2 = pool_w.tile([64, 1], F32)
            sq_b2 = pool_w.tile([64, 1], F32)
            sv_a2 = pool_w.tile([1, 64], F32)
            sv_b2 = pool_w.tile([1, 64], F32)
            nc.sync.dma_start(gvq_a2[:], cc_out[0:64, 64:128])
            nc.sync.dma_start(sq_a2[:], cc_out[128:129, 64:128])
            nc.sync.dma_start(sv_a2[:], cc_out[128:129, 0:64])
            nc.sync.dma_start(gvq_b2[:], cc_out_b[0:64, 64:128])
            nc.sync.dma_start(sq_b2[:], cc_out_b[128:129, 64:128])
            nc.sync.dma_start(sv_b2[:], cc_out_b[128:129, 0:64])
            gvq_sb = pool_w.tile([64, 64], F32)     # [d, c] = v_d . q_c
            sq_sb = pool_w.tile([64, 1], F32)
            sv_sb = pool_w.tile([1, 64], F32)
            nc.vector.tensor_add(gvq_sb[:], gvq_a2[:], gvq_b2[:])
            nc.vector.tensor_add(sq_sb[:], sq_a2[:], sq_b2[:])
            nc.vector.tensor_add(sv_sb[:], sv_a2[:], sv_b2[:])

            # ---------------- softmax + fused output weights ----------------
            with ExitStack() as p15:
                ps_sm = p15.enter_context(
                    tc.tile_pool(name="ps_sm", bufs=1, space="PSUM"))
                # rq = temp / max(sqrt(sq), eps)   [64, 1]
                nq = pool_w.tile([64, 1], F32)
                nc.scalar.sqrt(nq[:], sq_sb[:])
                nc.vector.tensor_scalar_max(nq[:], nq[:], EPS)
                rq = pool_w.tile([64, 1], F32)
                nc.vector.reciprocal(rq[:], nq[:])
                tb_ps = ps_sm.tile([64, 1], F32)
                nc.tensor.matmul(tb_ps[:], ones_sb[:], tp_sb[:],
                                 start=True, stop=True)
                nc.vector.tensor_mul(rq[:], rq[:], tb_ps[:])
                # rv row then broadcast to [64, 64] via ones-column matmul
                nv = pool_w.tile([1, 64], F32)
                nc.scalar.sqrt(nv[:], sv_sb[:])
                nc.vector.tensor_scalar_max(nv[:], nv[:], EPS)
                rv = pool_w.tile([1, 64], F32)
                nc.vector.reciprocal(rv[:], nv[:])
                rvb_ps = ps_sm.tile([64, 64], F32)
                nc.tensor.matmul(rvb_ps[:], ones_sb[:], rv[:],
                                 start=True, stop=True)
                # Gqv = Gvq^T
                gqv_ps = ps_sm.tile([64, 64], F32)
                nc.tensor.transpose(gqv_ps[:], gvq_sb[:], id_f32[:])
                gqv_sb = pool_w.tile([64, 64], F32)
                nc.vector.tensor_copy(gqv_sb[:], gqv_ps[:])
                # z = Gqv * rq[c] * rv[d]
                z = pool_w.tile([64, 64], F32)
                nc.vector.scalar_tensor_tensor(
                    out=z[:], in0=gqv_sb[:], scalar=rq[:], in1=rvb_ps[:],
                    op0=mybir.AluOpType.mult, op1=mybir.AluOpType.mult)
                mx = pool_w.tile([64, 1], F32)
                nc.vector.tensor_reduce(
                    mx[:], z[:], axis=mybir.AxisListType.X,
                    op=mybir.AluOpType.max)
                nc.vector.tensor_scalar(
                    out=z[:], in0=z[:], scalar1=mx[:], scalar2=None,
                    op0=mybir.AluOpType.subtract)
                e = pool_w.tile([64, 64], F32)
                sums = pool_w.tile([64, 1], F32)
                nc.scalar.activation(
                    e[:], z[:], mybir.ActivationFunctionType.Exp,
                    accum_out=sums[:])
                rs = pool_w.tile([64, 1], F32)
                nc.vector.reciprocal(rs[:], sums[:])
                attn = pool_w.tile([64, 64], F32)
                nc.vector.tensor_scalar(
                    out=attn[:], in0=e[:], scalar1=rs[:], scalar2=None,
                    op0=mybir.AluOpType.mult)
                # A2T = attn^T @ w_out^T  ->  [d, o]
                a2t_ps = ps_sm.tile([64, 64], F32)
                nc.tensor.matmul(a2t_ps[:], attn[:], wo_sb[:],
                                 start=True, stop=True)
                a2t_bf = pool_w.tile([64, 64], BF16)
                nc.vector.tensor_copy(a2t_bf[:], a2t_ps[:])

            # ---------------- pass 2: out = A2 @ v, streamed ----------------
            with ExitStack() as p2:
                ps_o = p2.enter_context(
                    tc.tile_pool(name="ps_o", bufs=3, space="PSUM"))
                ob_pool = p2.enter_context(tc.tile_pool(name="ob", bufs=3))
                BIG = 4096
                for T in range(N // BIG):
                    ob_sb = ob_pool.tile([128, BIG // 2], F32)
                    for j in range(4):
                        k = (BIG // NT) * T + 2 * j
                        ps = ps_o.tile([128, NT], F32)
                        nc.tensor.matmul(
                            ps[0:64, :], a2t_bf[:],
                            s_t[0:64, NT * k: NT * (k + 1)],
                            start=True, stop=True)
                        nc.tensor.matmul(
                            ps[64:128, :], a2t_bf[:],
                            s_t[0:64, NT * (k + 1): NT * (k + 2)],
                            start=True, stop=True, tile_position=(0, 64))
                        if j % 2 == 0:
                            nc.scalar.copy(
                                ob_sb[:, NT * j: NT * (j + 1)], ps[:])
                        else:
                            nc.vector.tensor_copy(
                                ob_sb[:, NT * j: NT * (j + 1)], ps[:])
                    dstv = out_d[0:64, BIG * T: BIG * (T + 1)].rearrange(
                        "c (j f) -> c j f", j=4, f=2 * NT)
                    srcv = ob_sb[:].rearrange("p (j f) -> p j f", j=4, f=NT)
                    nc.sync.dma_start(dstv[:, :, 0:NT], srcv[0:64])
                    nc.sync.dma_start(dstv[:, :, NT:2 * NT], srcv[64:128])

    nc.compile()
    return nc


def _get_nc():
    if "nc" not in _CACHE:
        _CACHE["nc"] = build_nc()
    return _CACHE["nc"]


def _prep_in_maps(x, w_in, w_dw, w_out, temperature):
    x = np.ascontiguousarray(x, dtype=np.float32)
    w_in = np.asarray(w_in, dtype=np.float32)
    w_dw = np.asarray(w_dw, dtype=np.float32)
    w_out = np.asarray(w_out, dtype=np.float32)
    temp = np.asarray(temperature, dtype=np.float32).reshape(1, 1)

    perm = np.concatenate([np.arange(64, 128), np.arange(0, 64)])
    W_in1 = w_in[:2 * C]          # [128, 64]
    W_in2 = w_in[2 * C:]          # [128, 64]
    wd = w_dw[:, 0]               # [128, 3, 3]

    import ml_dtypes
    bf = ml_dtypes.bfloat16

    wp = np.empty((128, 3 * 128), dtype=np.float32)
    for p, (ky, kx) in enumerate(PAIR_TAPS):
        wp[:64, 128 * p:128 * (p + 1)] = \
            (W_in1[perm] * wd[perm, ky, kx][:, None]).T
        wp[64:, 128 * p:128 * (p + 1)] = \
            (W_in1[perm] * wd[perm, ky + 1, kx + 1][:, None]).T
    # ws col block 0: K=128 pair [(2,0) @ rows 0-63, (0,2) @ rows 64-127]
    # (xd3 low half reads tap (2,0) at view (0,0), high half tap (0,2));
    # col block 1: single (2,2) @ rows 0-63 (xd3 low half, view (0,2))
    ws = np.zeros((128, 2 * 128), dtype=np.float32)
    ws[0:64, 0:128] = (W_in1[perm] * wd[perm, 2, 0][:, None]).T
    ws[64:128, 0:128] = (W_in1[perm] * wd[perm, 0, 2][:, None]).T
    ws[0:64, 128:256] = (W_in1[perm] * wd[perm, 2, 2][:, None]).T
    w2 = np.ascontiguousarray(W_in2[perm].T)        # [64, 128]
    wo = np.ascontiguousarray(w_out.T)              # [64, 64]
    wp = wp.astype(bf)
    ws = ws.astype(bf)
    w2 = w2.astype(bf)

    in_maps = []
    for core in range(8):
        sample, half = core // 2, core % 2
        h0 = half * 128
        xp = np.zeros((64, HROWS, RS), dtype=bf)
        lo, hi = max(h0 - 1, 0), min(h0 + 129, H)
        xp[:, lo - (h0 - 1): hi - (h0 - 1), 1:257] = x[sample, :, lo:hi, :]
        xpf = xp.reshape(64, XLEN)
        xpf_pad = np.pad(xpf, ((0, 0), (0, 520)))
        xd = np.concatenate(
            [xpf, xpf_pad[:, SHIFT:SHIFT + XLEN]], axis=0)
        xd3 = np.concatenate(
            [xpf_pad[:, 516:516 + XLEN], xpf_pad[:, 2:2 + XLEN]], axis=0)
        in_maps.append({
            "xd": np.ascontiguousarray(xd),
            "xd3": np.ascontiguousarray(xd3),
            "wp": wp, "ws": ws, "w2": w2, "wo": wo, "tp": temp,
        })
    return in_maps


def _assemble(results):
    out = np.empty((B, C, H, W), dtype=np.float32)
    for core in range(8):
        sample, half = core // 2, core % 2
        out[sample, :, half * 128: half * 128 + 128, :] = \
            results[core]["out"].reshape(C, 128, W)
    return out


def run(trace=False, trace_cores=None, **inputs):
    if trace:
        _install_ntff_hook()
    nc = _get_nc()
    in_maps = _prep_in_maps(**inputs)
    res = run_bass_kernel_spmd(nc, in_maps, core_ids=list(range(8)),
                               trace=trace, trace_cores=trace_cores)
    if trace and res.mean_exec_time_ns:
        print(f"mean exec {res.mean_exec_time_ns/1000:.1f}us, "
              f"max core {res.max_exec_time_core_id}")
    return _assemble(res.results), res.exec_time_ns


def kernel(**inputs) -> np.ndarray:
    out, _ = run(trace=False, **inputs)
    return out
